# revision 9
# baseline (speedup 1.0000x reference)
"""Relative-position attention (music-transformer style) on 8 trn2 cores.

Sharding: 1 head per core (H=8). Each core computes its head's attention and
a partial output projection (Wout row-shard); host sums partials + bout.

Device-side math per core (head h):
  qkT   = [Wq_h | Wk_h]^T-path: PSUM = Wqk^T-tiles @ xT            (d-major)
  qtT/qhT = qkT[0:64]*1/8 + rcb/rpb (per-partition bias)           (64,1536)
  A^T   = Wp10T^T @ qhT   rank-10 position coefficients            (10,1536)
  v     = xT-tiles^T @ Wv  (j-major)                               12x(128,96)
  band  = A_I @ pos10T[:, band]  -> DRAM, re-read sheared (stride 1662)
          = relative-shifted position logits (rank-10; the all-ones mask
          columns f>=10 contribute a per-row constant that cancels in softmax)
  logits = content (qtT_I^T @ kT) + shifted band
  expS  = exp(logits) -> bf16, accum_out = row sums
  expS -> DRAM stage -> DMA-transpose -> expST (j-major)
  outT  = sum_j v_j^T @ expST_j                                    (96,1536)
  final_I = outT_I^T @ Wout * (1/sums_I)  -> out (1536,768) partial
"""

import sys

sys.path.insert(0, "/opt/trn_rl_repo")

import numpy as np
import ml_dtypes

import concourse.bass as bass
import concourse.tile as tile
from concourse import bacc, mybir
from concourse.bass_utils import run_bass_kernel_spmd
from concourse.tile_rust import add_dep_helper

F32 = mybir.dt.float32
BF16 = mybir.dt.bfloat16
AF = mybir.ActivationFunctionType

L, C, H, DK, DV = 1536, 768, 8, 64, 96
NT = L // 128          # 12 row tiles
NC = C // 128          # 6 contraction tiles
BAND = L + 127         # 1663
R10 = 10
P2N = 2 * L - 1        # 3071

LAST_RESULTS = None    # test.py reads exec_time_ns from here


def _build_nc():
    nc = bacc.Bacc(
        "TRN2",
        target_bir_lowering=False,
        debug=False,
        enable_asserts=True,
        num_devices=8,
    )
    xT = nc.dram_tensor("xT", [C, L], F32, kind="ExternalInput").ap()
    Wqk = nc.dram_tensor("Wqk", [C, 128], F32, kind="ExternalInput").ap()
    Wv = nc.dram_tensor("Wv", [C, DV], F32, kind="ExternalInput").ap()
    Wp10T = nc.dram_tensor("Wp10T", [DK, R10], F32, kind="ExternalInput").ap()
    pos10T = nc.dram_tensor("pos10T", [R10, P2N], BF16, kind="ExternalInput").ap()
    rcb = nc.dram_tensor("rcb", [DK, 1], F32, kind="ExternalInput").ap()
    rpb = nc.dram_tensor("rpb", [DK, 1], F32, kind="ExternalInput").ap()
    Wout = nc.dram_tensor("Wout", [DV, C], F32, kind="ExternalInput").ap()
    out = nc.dram_tensor("out", [L, C], F32, kind="ExternalOutput").ap()

    band_h = nc.dram_tensor("bandbuf", [NT, 128, BAND], BF16, kind="Internal")
    stage_h = nc.dram_tensor("stagebuf", [L, L], BF16, kind="Internal")
    band = band_h.ap()
    stage = stage_h.ap()

    with tile.TileContext(nc) as tc:
      with tc.tile_pool(name="consts", bufs=1) as consts:
        if True:
            # ---- persistent SBUF loads ----
            xT_sb = []
            Wqk_sb = []
            Wv_sb = []
            for c in range(NC):
                t = consts.tile([128, L], F32, tag=f"xT{c}")
                nc.sync.dma_start(t[:], xT[c * 128:(c + 1) * 128, :])
                xT_sb.append(t)
                w = consts.tile([128, 128], F32, tag=f"Wqk{c}")
                nc.sync.dma_start(w[:], Wqk[c * 128:(c + 1) * 128, :])
                Wqk_sb.append(w)
                w2 = consts.tile([128, DV], F32, tag=f"Wv{c}")
                nc.sync.dma_start(w2[:], Wv[c * 128:(c + 1) * 128, :])
                Wv_sb.append(w2)
            Wp10T_sb = consts.tile([DK, R10], F32, tag="Wp10T")
            nc.sync.dma_start(Wp10T_sb[:], Wp10T)
            pos10T_sb = consts.tile([R10, P2N], BF16, tag="pos10T")
            nc.sync.dma_start(pos10T_sb[:], pos10T)
            rcb_sb = consts.tile([DK, 1], F32, tag="rcb")
            nc.sync.dma_start(rcb_sb[:], rcb)
            rpb_sb = consts.tile([DK, 1], F32, tag="rpb")
            nc.sync.dma_start(rpb_sb[:], rpb)
            Wout_sb = consts.tile([DV, C], F32, tag="Wout")
            nc.sync.dma_start(Wout_sb[:], Wout)

            qtT = consts.tile([DK, L], F32, tag="qtT")
            qhT = consts.tile([DK, L], F32, tag="qhT")
            kT = consts.tile([DK, L], F32, tag="kT")
            AT = consts.tile([R10, L], BF16, tag="AT")
            outT = consts.tile([DV, L], F32, tag="outT")
            sums = consts.tile([128, NT], F32, tag="sums")
            invs = consts.tile([128, NT], F32, tag="invs")

            # ---- q/k projections: qkT (128,1536) = Wqk^T @ x^T ----
            ph1 = tc.tile_pool(name="ps_qk", bufs=2, space="PSUM")
            ps_qk = ph1.__enter__()
            ph1a = tc.tile_pool(name="ps_a", bufs=1, space="PSUM")
            ps_a = ph1a.__enter__()
            ph1v = tc.tile_pool(name="ps_v", bufs=2, space="PSUM")
            ps_v = ph1v.__enter__()
            for n in range(3):
                ps = ps_qk.tile([128, 512], F32, tag="ps_qk")
                sl = bass.ts(n, 512)
                for c in range(NC):
                    nc.tensor.matmul(
                        ps[:], Wqk_sb[c][:], xT_sb[c][:, sl],
                        start=(c == 0), stop=(c == NC - 1),
                    )
                nc.scalar.activation(qtT[:, sl], ps[0:DK, :], AF.Identity,
                                     bias=rcb_sb[:], scale=0.125)
                nc.scalar.activation(qhT[:, sl], ps[0:DK, :], AF.Identity,
                                     bias=rpb_sb[:], scale=0.125)
                nc.scalar.copy(kT[:, sl], ps[DK:128, :])

            # ---- rank-10 position coefficients A^T = Wp10T^T @ qhT ----
            for n in range(3):
                ps = ps_a.tile([R10, 512], F32, tag="ps_a")
                sl = bass.ts(n, 512)
                nc.tensor.matmul(ps[:], Wp10T_sb[:], qhT[:, sl],
                                 start=True, stop=True)
                nc.scalar.copy(AT[:, sl], ps[:])

            # ---- v (j-major): v_J = xT_J^T @ Wv ----
            v_sb = []
            for j in range(NT):
                ps = ps_v.tile([128, DV], F32, tag="ps_v")
                for c in range(NC):
                    nc.tensor.matmul(
                        ps[:], xT_sb[c][:, bass.ts(j, 128)], Wv_sb[c][:],
                        start=(c == 0), stop=(c == NC - 1),
                    )
                vt = consts.tile([128, DV], BF16, tag=f"v{j}")
                nc.scalar.copy(vt[:], ps[:])
                v_sb.append(vt)
            ph1v.__exit__(None, None, None)
            ph1a.__exit__(None, None, None)
            ph1.__exit__(None, None, None)

        # ---- band: position logits (unshifted, banded), to DRAM ----
        band_writes = []
        with (
            tc.tile_pool(name="band_ps", bufs=2, space="PSUM") as band_ps,
            tc.tile_pool(name="band_sb", bufs=3) as band_sb,
        ):
            for i in range(NT):
                lo = 1408 - 128 * i
                ps = band_ps.tile([128, BAND], F32, tag="ps_t")
                for n in range(4):
                    w = min(512, BAND - n * 512)
                    nc.tensor.matmul(ps[:, n * 512:n * 512 + w],
                                     AT[:, bass.ts(i, 128)],
                                     pos10T_sb[:, lo + n * 512:lo + n * 512 + w],
                                     start=True, stop=True)
                tb = band_sb.tile([128, BAND], BF16, tag="tband")
                nc.scalar.copy(tb[:], ps[:])
                w = nc.sync.dma_start(band[i], tb[:])
                band_writes.append(w)

        # ---- content + shear-read + add + exp + stage ----
        stage_writes = []
        with (
            tc.tile_pool(name="cont_ps", bufs=2, space="PSUM") as cont_ps,
            tc.tile_pool(name="smax_sb", bufs=2) as smax_sb,
        ):
            for i in range(NT):
                ps = cont_ps.tile([128, L], F32, tag="ps_c")
                for n in range(3):
                    nc.tensor.matmul(ps[:, bass.ts(n, 512)],
                                     qtT[:, bass.ts(i, 128)],
                                     kT[:, bass.ts(n, 512)],
                                     start=True, stop=True)
                shifted = smax_sb.tile([128, L], BF16, tag="shifted")
                shear = bass.AP(band_h, i * 128 * BAND + 127,
                                [[BAND - 1, 128], [1, L]])
                r = nc.sync.dma_start(shifted[:], shear)
                add_dep_helper(r.ins, band_writes[i].ins, reason="band RAW")
                logits = smax_sb.tile([128, L], F32, tag="logits")
                nc.vector.tensor_add(logits[:], ps[:], shifted[:])
                expS = smax_sb.tile([128, L], BF16, tag="expS")
                nc.scalar.activation(expS[:], logits[:], AF.Exp,
                                     accum_out=sums[:, i:i + 1])
                w = nc.sync.dma_start(stage[bass.ts(i, 128), :], expS[:])
                stage_writes.append(w)

        # ---- transpose + attn@v: outT = sum_j v_j^T @ expST_j ----
        with (
            tc.tile_pool(name="av_ps", bufs=1, space="PSUM") as av_ps,
            tc.tile_pool(name="avT_sb", bufs=3) as avT_sb,
        ):
            ps_o = av_ps.tile([DV, L], F32, tag="ps_o")
            for j in range(NT):
                expT = avT_sb.tile([128, L], BF16, tag="expT")
                r = nc.sync.dma_start(expT[:], stage[:, bass.ts(j, 128)],
                                      transpose=True)
                for wi in stage_writes:
                    add_dep_helper(r.ins, wi.ins, reason="stage RAW")
                for n in range(3):
                    nc.tensor.matmul(ps_o[:, bass.ts(n, 512)],
                                     v_sb[j][:],
                                     expT[:, bass.ts(n, 512)],
                                     start=(j == 0), stop=(j == NT - 1))
            nc.scalar.copy(outT[:], ps_o[:])
            nc.vector.reciprocal(invs[:], sums[:])

        # ---- final projection + normalize ----
        with (
            tc.tile_pool(name="fin_ps", bufs=2, space="PSUM") as fin_ps,
            tc.tile_pool(name="fin_sb", bufs=3) as fin_sb,
        ):
            for i in range(NT):
                ps = fin_ps.tile([128, C], F32, tag="ps_f")
                nc.tensor.matmul(ps[:, 0:512], outT[:, bass.ts(i, 128)],
                                 Wout_sb[:, 0:512], start=True, stop=True)
                nc.tensor.matmul(ps[:, 512:C], outT[:, bass.ts(i, 128)],
                                 Wout_sb[:, 512:C], start=True, stop=True)
                fin = fin_sb.tile([128, C], F32, tag="fin")
                nc.scalar.activation(fin[:], ps[:], AF.Copy,
                                     scale=invs[:, i:i + 1])
                nc.sync.dma_start(out[bass.ts(i, 128), :], fin[:])

    return nc


def _host_pos10():
    p = np.arange(-(L - 1), L, dtype=np.float32)
    w = (2.0 ** np.arange(1, R10 + 1, dtype=np.float32)) - 1.0
    m = (w[None, :] > np.abs(p)[:, None]).astype(np.float32)  # (3071, 10)
    return np.ascontiguousarray(m.T).astype(ml_dtypes.bfloat16)


def kernel(x, Wq, Wk, Wv, Wout, bout, Wpos, rel_content_bias, rel_pos_bias):
    global LAST_RESULTS
    x = np.asarray(x, np.float32)
    Wq, Wk, Wv = (np.asarray(a, np.float32) for a in (Wq, Wk, Wv))
    Wout, bout, Wpos = (np.asarray(a, np.float32) for a in (Wout, bout, Wpos))
    rcb = np.asarray(rel_content_bias, np.float32)[0, :, 0, :]  # (8,64)
    rpb = np.asarray(rel_pos_bias, np.float32)[0, :, 0, :]

    xT = np.ascontiguousarray(x[0].T)          # (768, 1536)
    pos10T = _host_pos10()                      # (10, 3071) bf16

    in_maps = []
    for h in range(H):
        qs, vs = slice(h * DK, (h + 1) * DK), slice(h * DV, (h + 1) * DV)
        in_maps.append({
            "xT": xT,
            "Wqk": np.ascontiguousarray(
                np.concatenate([Wq[:, qs], Wk[:, qs]], axis=1)),
            "Wv": np.ascontiguousarray(Wv[:, vs]),
            "Wp10T": np.ascontiguousarray(Wpos[:R10, qs].T),
            "pos10T": pos10T,
            "rcb": np.ascontiguousarray(rcb[h][:, None]),
            "rpb": np.ascontiguousarray(rpb[h][:, None]),
            "Wout": np.ascontiguousarray(Wout[vs, :]),
        })

    nc = _build_nc()
    nc.finalize()
    res = run_bass_kernel_spmd(nc, in_maps, core_ids=list(range(H)))
    LAST_RESULTS = res
    total = np.zeros((L, C), np.float32)
    for r in res.results:
        total += np.asarray(r["out"], np.float32)
    total += bout
    return total[None, :, :]


if __name__ == "__main__":
    d = np.load("/root/problem/ref_cache.npz")
    inputs = {k: d[k] for k in ["x", "Wq", "Wk", "Wv", "Wout", "bout", "Wpos",
                                "rel_content_bias", "rel_pos_bias"]}
    got = kernel(**inputs)
    ref = d["ref"]
    rel = np.linalg.norm(got - ref) / np.linalg.norm(ref)
    print("Relative error:", rel)
    if LAST_RESULTS is not None and LAST_RESULTS.exec_time_ns:
        print("HW exec time:", LAST_RESULTS.exec_time_ns, "ns")
